# revision 8
# baseline (speedup 1.0000x reference)
"""Trainium2 Bass kernel for nn_MixedAttnHeadEmbed_82076825027210.

Computes, per batch element:
    out = sum over h in {4, 8, 12} of CausalAttention(Q_mix_h, K_mix_h, V_mix_h)
where Q/K/V_mix_h are weighted mixtures (9 scalar weights) of head-sliced
views of x's q/k/v channel groups, padded per head to hd = 768/h.

Sharding: data-parallel over batch B=8 across the 8 NeuronCores (one batch
element per core); the 9 mixture weights are baked into the compiled program
as immediates.

Per-core engine assignment (engine-busy budget vs the ~119us ACT floor):
  ACT  exp only -- the hard floor: 92us of element time + per-instr init
  PE   S^T chunks, diagonal-mask matmuls, PV (+l via ones column)
  DVE  Q/K/V mixing (tensor_scalar@4x + tensor_tensor adds@2x)
  Pool normalize: rec = ones/l via tensor_tensor divide (keeps the in-order
       DVE queue free of sem-waiting ops so mixing streams ahead),
       scalar_tensor_tensor PSUM->oacc accumulate, small memsets, x-load
       SWDGE prep
  DMA  x f32->bf16 cast loads, DRAM bounce + 16x128-tile DMA transposes of
       the mixed Q/K naturals, per-query-block output stores

Schedule: configs processed h=12 -> h=8 -> h=4 so the first config's exp
stream (the longest) provides runway to mix/bounce/transpose everything
else behind it; per config passes run hf-outer/s-inner so half-1 operands
are needed as late as possible.  One software-pipelined attention stream
across all 3 configs (S^T+exp of job i, then PV of job i-1, crossing pass
and config boundaries); the driver pumps the next config's DVE mixing
between attention jobs.  x half-1 loads carry an explicit dep on the K
half-0 bounce so they don't cut ahead of the startup-critical transposes
on the FIFO DMA device.
"""

import math
from collections import deque

import numpy as np

import concourse.bass as bass
import concourse.bacc as bacc
import concourse.tile as tile
from concourse import mybir
from concourse.bass_utils import run_bass_kernel_spmd
from concourse.tile import add_dep_helper

F32 = mybir.dt.float32
BF16 = mybir.dt.bfloat16
ALU = mybir.AluOpType
ACTF = mybir.ActivationFunctionType

T = 1024
NT = 8  # token tiles of 128
E = 768
CIN = 3 * E
N_HEAD_LIST = (4, 8, 12)
CFG_ORDER = (2, 1, 0)  # process h=12 first: longest exp runway
N_CORES = 8
MASK_NEG = -3000.0  # additive pre-scale mask; exp(scale*MASK_NEG) == 0


def _pw(h):
    """Per-head column pitch in the natural mixed layout; h=8 pads 96 -> 128
    so every transposed head starts at a legal matmul base partition."""
    return 128 if h == 8 else E // h


def _dchunks(h):
    """Per head: contraction (d) row ranges in the transposed layout, split
    at 128-row tile boundaries."""
    hd = E // h
    pitch = _pw(h)
    out = []
    for i in range(h):
        a, b = i * pitch, i * pitch + hd
        chunks = []
        while a < b:
            nxt = min(b, (a // 128 + 1) * 128)
            chunks.append((a, nxt))
            a = nxt
        out.append(chunks)
    return out


def _build_program(W):
    """W: numpy [9] f32 mixture weights. Returns compiled Bacc program."""
    nc = bacc.Bacc(
        "TRN2", target_bir_lowering=False, debug=False, num_devices=N_CORES
    )
    x_in = nc.dram_tensor("x", [T, CIN], F32, kind="ExternalInput").ap()
    out_d = nc.dram_tensor("out", [T, E], F32, kind="ExternalOutput").ap()
    qk_dram = [
        [
            nc.dram_tensor(
                f"qkb_{ci}_{ti}", [T, N_HEAD_LIST[ci] * _pw(N_HEAD_LIST[ci])],
                BF16,
            ).ap()
            for ti in range(2)
        ]
        for ci in range(3)
    ]

    with tile.TileContext(nc) as tc:
        _emit(tc, x_in, out_d, qk_dram, W)
    nc.compile()
    return nc


def _emit(tc, x_in, out_d, qk_dram, W):
    nc = tc.nc
    with (
        tc.tile_pool(name="consts", bufs=1) as consts,
        tc.tile_pool(name="xbf", bufs=1) as xbf_pool,
        tc.tile_pool(name="nat", bufs=2) as nat_pool,
        tc.tile_pool(name="tmp", bufs=1) as tmp_pool,
        tc.tile_pool(name="qkt", bufs=2) as qkt_pool,
        tc.tile_pool(name="vaug", bufs=2) as vaug_pool,
        tc.tile_pool(name="pt", bufs=6) as pt_pool,
        tc.tile_pool(name="small", bufs=4) as small_pool,
        tc.tile_pool(name="oacc", bufs=1) as oacc_pool,
        tc.tile_pool(name="stage", bufs=2, space="PSUM") as stage_pool,
        tc.tile_pool(name="ypsum", bufs=4, space="PSUM") as ypsum_pool,
    ):
        xbf = xbf_pool.tile([128, NT, CIN], BF16)

        def load_x_chunk(third, half):
            c0 = third * E + half * (E // 2)
            return nc.gpsimd.dma_start(
                out=xbf[:, :, c0 : c0 + E // 2],
                in_=x_in[:, c0 : c0 + E // 2].rearrange(
                    "(a p) c -> p a c", p=128
                ),
            )

        # startup: q/k/v half-0 chunks first (q,k feed the critical mixes)
        load_x_chunk(0, 0)
        load_x_chunk(1, 0)
        load_x_chunk(2, 0)

        # ---- constants: strict-upper selector, MASK_NEG * I, ones row ---
        ustrict = consts.tile([128, 128], BF16)
        nc.gpsimd.memset(ustrict, 1.0)
        nc.gpsimd.affine_select(
            out=ustrict, in_=ustrict, compare_op=ALU.is_gt, fill=0.0,
            base=0, pattern=[[1, 128]], channel_multiplier=-1,
        )
        negi = consts.tile([128, 128], BF16)
        nc.gpsimd.memset(negi, 0.0)
        nc.gpsimd.affine_select(
            out=negi, in_=negi, compare_op=ALU.not_equal, fill=MASK_NEG,
            base=0, pattern=[[-1, 128]], channel_multiplier=1,
        )
        onesf = consts.tile([128, 8], F32)
        nc.gpsimd.memset(onesf, 1.0)

        oacc = oacc_pool.tile([128, NT, E], F32)

        state = {}

        # weight order in W: for cfg ci, e in (384, 576, 768): W[3*ci + idx]
        def mix_config(oi):
            """Generator. Emits DVE mixing + bounce/transpose DMAs for one
            config (order index oi), yielding between DVE ops.  Yields
            "ready" (oi==0 only) once attention may start."""
            ci = CFG_ORDER[oi]
            h = N_HEAD_LIST[ci]
            hd = E // h
            pw = _pw(h)
            h2 = h // 2
            e_list = [(2, 768, hd), (1, 576, 576 // h), (0, 384, 384 // h)]
            ndt = h * pw // 128
            ndt2 = ndt // 2

            qkt = []
            vaug = vaug_pool.tile([128, NT, h, hd + 1], BF16, tag="vaug")
            for tensor_idx in range(2):
                tl = qkt_pool.tile(
                    [128, ndt, T], BF16, tag="qkt", bufs=4,
                    name=f"qkt{ci}{tensor_idx}",
                )
                qkt.append(tl)
            tmp = tmp_pool.tile([128, NT, 288], BF16, tag="tmp")
            tmpb = tmp_pool.tile([128, NT, 288], BF16, tag="tmpb")
            state[ci] = (qkt, vaug)

            def mix_into(out_ap, xsrc, tmps):
                """Yields after each DVE op. out_ap(hde) is the dest slice,
                xsrc(e, hde) the source slice for mixture term e."""
                for idx, (k, e, hde) in enumerate(e_list):
                    w = float(W[3 * ci + k])
                    in0 = xsrc(e, hde)
                    if idx == 0:
                        nc.vector.tensor_scalar(
                            out_ap(hde), in0, w, None, ALU.mult
                        )
                        yield
                    else:
                        tview = tmps[idx % len(tmps)].rearrange(
                            "p a (h d) -> p a h d", h=h2
                        )
                        tv = tview[:, :, :, 0:hde]
                        nc.vector.tensor_scalar(tv, in0, w, None, ALU.mult)
                        yield
                        nc.vector.tensor_tensor(
                            out_ap(hde), tv, out_ap(hde), ALU.add
                        )
                        yield

            k0_bounce = [None]
            for half in range(2):
                if oi == 0 and half == 1:
                    # half-1 x loads: hold behind the K half-0 bounce so
                    # they don't cut ahead on the FIFO DMA device
                    for third in range(3):
                        ld = load_x_chunk(third, 1)
                        if k0_bounce[0] is not None:
                            add_dep_helper(
                                ld.ins, k0_bounce[0].ins, sync=True,
                                reason="x h1 after startup bounces",
                            )
                hsl = slice(half * h2, (half + 1) * h2)
                for tensor_idx in range(2):
                    base = tensor_idx * E
                    nat = nat_pool.tile([128, NT, h2, pw], BF16, tag="nat")
                    if pw > hd:
                        nc.vector.memset(nat[:, :, :, hd:pw], 0.0)

                    def xsrc(e, hde, base=base, half=half):
                        sl = xbf[
                            :, :,
                            base + half * (e // 2)
                            : base + (half + 1) * (e // 2),
                        ]
                        return sl.rearrange("p a (h d) -> p a h d", h=h2)

                    def out_ap(hde, nat=nat):
                        return nat[:, :, :, 0:hde]

                    for _ in mix_into(out_ap, xsrc, (tmp, tmpb)):
                        yield

                    # bounce to DRAM + one 3D transpose read.  The startup-
                    # critical K half-0 chain of the first config rides the
                    # ACT ring (idle until the first exp, which depends on
                    # this transpose anyway) so it doesn't serialize behind
                    # the Q chain on SP; everything else stays on SP where
                    # DMA waits can't stall the exp stream's sequencer.
                    startup_k = oi == 0 and half == 0 and tensor_idx == 1
                    eng = nc.scalar if startup_k else nc.sync
                    w0 = half * h2 * pw
                    wr = eng.dma_start(
                        out=qk_dram[ci][tensor_idx][
                            :, w0 : w0 + h2 * pw
                        ].rearrange("(a p) w -> p a w", p=128),
                        in_=nat[:, :, :, :],
                    )
                    rd = eng.dma_start(
                        out=qkt[tensor_idx][
                            :, half * ndt2 : (half + 1) * ndt2, :
                        ],
                        in_=qk_dram[ci][tensor_idx][:, w0 : w0 + h2 * pw],
                        transpose=True,
                    )
                    add_dep_helper(
                        rd.ins, wr.ins, sync=True, reason="dram bounce raw"
                    )
                    if half == 0 and tensor_idx == 1:
                        k0_bounce[0] = wr
                    yield

                # V_aug for this half
                nc.gpsimd.memset(vaug[:, :, hsl, hd : hd + 1], 1.0)

                def vsrc(e, hde, half=half):
                    sl = xbf[
                        :, :,
                        2 * E + half * (e // 2)
                        : 2 * E + (half + 1) * (e // 2),
                    ]
                    return sl.rearrange("p a (h d) -> p a h d", h=h2)

                def vout(hde, hsl=hsl):
                    return vaug[:, :, hsl, 0:hde]

                for _ in mix_into(vout, vsrc, (tmp, tmpb)):
                    yield
                if oi == 0 and half == 0:
                    yield "ready"

        def attention():
            """Single software-pipelined job stream across all 3 configs."""
            prev = [None]  # carried (emit_fn, tk, g, ptl) across passes

            for oi, ci in enumerate(CFG_ORDER):
                if oi > 0:
                    yield ("cfg", oi)
                h = N_HEAD_LIST[ci]
                hd = E // h
                h2 = h // 2
                scale = 1.0 / math.sqrt(hd)
                dchunks = _dchunks(h)
                qkt, vaug = state[ci]
                qt, kt = qkt

                for hf in range(2):
                    for s in range(2):
                        ntk = 4 * s + 4
                        pheads = list(range(hf * h2, (hf + 1) * h2))
                        nh = h2
                        groups = [
                            pheads[i : i + 2] for i in range(0, nh, 2)
                        ]
                        yts = [
                            ypsum_pool.tile(
                                [128, nh, hd + 1], F32, tag="y",
                                name=f"yt{ci}{s}{hf}{qt_}",
                            )
                            for qt_ in range(4)
                        ]
                        y_first = [None] * 4

                        def norm_qt(qt_, *, oi=oi, s=s, hf=hf, yts=yts,
                                    pheads=pheads, nh=nh, hd=hd):
                            tqg = 4 * s + qt_
                            rec = small_pool.tile([128, 6], F32, tag="rec")
                            # last pass: DVE is idle (all mixing done) and
                            # Pool serialization would stretch the tail
                            veng = nc.vector if (oi == 2 and hf == 1) \
                                else nc.gpsimd
                            veng.tensor_tensor(
                                rec[:, 0:nh], onesf[:, 0:nh],
                                yts[qt_][:, :, hd], ALU.divide,
                            )
                            for jp, head in enumerate(pheads):
                                dst = oacc[
                                    :, tqg, head * hd : head * hd + hd
                                ]
                                if oi == 0:
                                    veng.tensor_scalar(
                                        dst, yts[qt_][:, jp, 0:hd],
                                        rec[:, jp : jp + 1], None, ALU.mult,
                                    )
                                else:
                                    veng.scalar_tensor_tensor(
                                        out=dst,
                                        in0=yts[qt_][:, jp, 0:hd],
                                        scalar=rec[:, jp : jp + 1],
                                        in1=dst,
                                        op0=ALU.mult,
                                        op1=ALU.add,
                                    )
                            if oi == 2 and hf == 1:
                                # this query tile is final: stream out
                                nc.sync.dma_start(
                                    out=out_d[tqg * 128 : (tqg + 1) * 128, :],
                                    in_=oacc[:, tqg, :],
                                )

                        def emit_pv(tk, g, ptl, *, s=s, hf=hf, nh=nh, hd=hd,
                                    yts=yts, y_first=y_first, vaug=vaug,
                                    groups=groups, norm_qt=norm_qt):
                            for qt_ in range(4):
                                qtg = 4 * s + qt_
                                if qtg < tk:
                                    continue
                                for j, head in enumerate(g):
                                    jp = head - hf * nh
                                    is_start = (
                                        tk == 0 and y_first[qt_] is None
                                    )
                                    mm = nc.tensor.matmul(
                                        out=yts[qt_][:, jp, :],
                                        lhsT=ptl[
                                            :, j, qt_ * 128 : (qt_ + 1) * 128
                                        ],
                                        rhs=vaug[:, tk, head, :],
                                        start=is_start,
                                        stop=(tk == qtg and jp == nh - 1),
                                    )
                                    if is_start:
                                        y_first[qt_] = mm
                                    elif tk == 0:
                                        add_dep_helper(
                                            mm.ins,
                                            y_first[qt_].ins,
                                            reason="psum zero-region order",
                                        )
                            if g is groups[-1] and 0 <= tk - 4 * s < 4:
                                norm_qt(tk - 4 * s)

                        for tk in range(ntk):
                            lo = max(0, tk * 128 - s * 512)
                            diag = tk >= 4 * s
                            dlo = tk * 128 - s * 512
                            for g in groups:
                                stage = stage_pool.tile(
                                    [128, 2, 512], F32, tag="stage"
                                )
                                for j, head in enumerate(g):
                                    chunks = dchunks[head]
                                    n_mm = len(chunks) + (1 if diag else 0)
                                    for mi, (a, b) in enumerate(chunks):
                                        nc.tensor.matmul(
                                            out=stage[:, j, lo:512],
                                            lhsT=kt[
                                                a % 128 : a % 128 + (b - a),
                                                a // 128,
                                                tk * 128 : (tk + 1) * 128,
                                            ],
                                            rhs=qt[
                                                a % 128 : a % 128 + (b - a),
                                                a // 128,
                                                s * 512 + lo : (s + 1) * 512,
                                            ],
                                            start=(mi == 0),
                                            stop=(mi == n_mm - 1),
                                        )
                                    if diag:
                                        nc.tensor.matmul(
                                            out=stage[:, j, dlo : dlo + 128],
                                            lhsT=ustrict[:, :],
                                            rhs=negi[:, :],
                                            start=False,
                                            stop=True,
                                        )
                                ptl = pt_pool.tile(
                                    [128, 2, 512], BF16, tag="pt"
                                )
                                nc.scalar.activation(
                                    out=ptl[:, 0:2, lo:512],
                                    in_=stage[:, 0:2, lo:512],
                                    func=ACTF.Exp,
                                    scale=scale,
                                )
                                if prev[0] is not None:
                                    pfn, ptk, pg, pptl = prev[0]
                                    pfn(ptk, pg, pptl)
                                prev[0] = (emit_pv, tk, g, ptl)
                                yield
            if prev[0] is not None:
                pfn, ptk, pg, pptl = prev[0]
                pfn(ptk, pg, pptl)

        # ---- driver: startup mix, then attention with mix pumping ------
        gens = deque([(oi, mix_config(oi)) for oi in range(3)])
        g0 = gens[0][1]
        while True:
            if next(g0) == "ready":
                break

        def pump(n):
            for _ in range(n):
                while gens:
                    try:
                        next(gens[0][1])
                        break
                    except StopIteration:
                        gens.popleft()
                else:
                    return

        def drain_through(oi):
            while gens and gens[0][0] <= oi:
                try:
                    next(gens[0][1])
                except StopIteration:
                    gens.popleft()

        for item in attention():
            if isinstance(item, tuple) and item[0] == "cfg":
                drain_through(item[1])
            else:
                pump(2)
        while gens:
            try:
                next(gens[0][1])
            except StopIteration:
                gens.popleft()


_PROGRAM_CACHE = {}


def _get_program(W):
    key = np.asarray(W, dtype=np.float32).tobytes()
    if key not in _PROGRAM_CACHE:
        _PROGRAM_CACHE[key] = _build_program(np.asarray(W, dtype=np.float32))
    return _PROGRAM_CACHE[key]


def kernel(x, weights):
    """x: [8, 1024, 2304] f32; weights: [9] f32 -> [8, 1024, 768] f32."""
    x = np.asarray(x, dtype=np.float32)
    weights = np.asarray(weights, dtype=np.float32)
    assert x.shape == (N_CORES, T, CIN), x.shape
    nc = _get_program(weights)
    in_maps = [{"x": np.ascontiguousarray(x[c])} for c in range(N_CORES)]
    res = run_bass_kernel_spmd(nc, in_maps, list(range(N_CORES)))
    return np.stack([res.results[c]["out"] for c in range(N_CORES)], axis=0)


# revision 11
# speedup vs baseline: 1.0240x; 1.0240x over previous
"""Trainium2 Bass kernel for nn_MixedAttnHeadEmbed_82076825027210.

Computes, per batch element:
    out = sum over h in {4, 8, 12} of CausalAttention(Q_mix_h, K_mix_h, V_mix_h)
where Q/K/V_mix_h are weighted mixtures (9 scalar weights) of head-sliced
views of x's q/k/v channel groups, padded per head to hd = 768/h.

Sharding: data-parallel over batch B=8 across the 8 NeuronCores (one batch
element per core); the 9 mixture weights are baked into the compiled program
as immediates.

Per-core engine assignment (engine-busy budget vs the ~119us ACT floor):
  ACT  exp only -- the hard floor: 92us of element time + per-instr init
  PE   S^T chunks, diagonal-mask matmuls, PV (+l via ones column)
  DVE  Q/K/V mixing (tensor_scalar@4x + tensor_tensor adds@2x)
  Pool normalize: rec = ones/l via tensor_tensor divide (keeps the in-order
       DVE queue free of sem-waiting ops so mixing streams ahead),
       scalar_tensor_tensor PSUM->oacc accumulate, small memsets, x-load
       SWDGE prep
  DMA  x f32->bf16 cast loads, DRAM bounce + 16x128-tile DMA transposes of
       the mixed Q/K naturals, per-query-block output stores

Schedule: configs processed h=12 -> h=8 -> h=4 so the first config's exp
stream (the longest) provides runway to mix/bounce/transpose everything
else behind it; per config passes run hf-outer/s-inner so half-1 operands
are needed as late as possible.  One software-pipelined attention stream
across all 3 configs (S^T+exp of job i, then PV of job i-1, crossing pass
and config boundaries); the driver pumps the next config's DVE mixing
between attention jobs.  x half-1 loads carry an explicit dep on the K
half-0 bounce so they don't cut ahead of the startup-critical transposes
on the FIFO DMA device.
"""

import math
from collections import deque

import numpy as np

import concourse.bass as bass
import concourse.bacc as bacc
import concourse.tile as tile
from concourse import mybir
from concourse.bass_utils import run_bass_kernel_spmd
from concourse.tile import add_dep_helper

F32 = mybir.dt.float32
BF16 = mybir.dt.bfloat16
ALU = mybir.AluOpType
ACTF = mybir.ActivationFunctionType

T = 1024
NT = 8  # token tiles of 128
E = 768
CIN = 3 * E
N_HEAD_LIST = (4, 8, 12)
CFG_ORDER = (2, 1, 0)  # process h=12 first: longest exp runway
N_CORES = 8
MASK_NEG = -3000.0  # additive pre-scale mask; exp(scale*MASK_NEG) == 0


def _pw(h):
    """Per-head column pitch in the natural mixed layout; h=8 pads 96 -> 128
    so every transposed head starts at a legal matmul base partition."""
    return 128 if h == 8 else E // h


def _dchunks(h):
    """Per head: contraction (d) row ranges in the transposed layout, split
    at 128-row tile boundaries."""
    hd = E // h
    pitch = _pw(h)
    out = []
    for i in range(h):
        a, b = i * pitch, i * pitch + hd
        chunks = []
        while a < b:
            nxt = min(b, (a // 128 + 1) * 128)
            chunks.append((a, nxt))
            a = nxt
        out.append(chunks)
    return out


def _build_program(W):
    """W: numpy [9] f32 mixture weights. Returns compiled Bacc program."""
    nc = bacc.Bacc(
        "TRN2", target_bir_lowering=False, debug=False, num_devices=N_CORES
    )
    x_in = nc.dram_tensor("x", [T, CIN], F32, kind="ExternalInput").ap()
    out_d = nc.dram_tensor("out", [T, E], F32, kind="ExternalOutput").ap()
    qk_dram = [
        [
            nc.dram_tensor(
                f"qkb_{ci}_{ti}", [T, N_HEAD_LIST[ci] * _pw(N_HEAD_LIST[ci])],
                BF16,
            ).ap()
            for ti in range(2)
        ]
        for ci in range(3)
    ]

    with tile.TileContext(nc) as tc:
        _emit(tc, x_in, out_d, qk_dram, W)
    nc.compile()
    return nc


def _emit(tc, x_in, out_d, qk_dram, W):
    nc = tc.nc
    with (
        tc.tile_pool(name="consts", bufs=1) as consts,
        tc.tile_pool(name="xbf", bufs=1) as xbf_pool,
        tc.tile_pool(name="nat", bufs=2) as nat_pool,
        tc.tile_pool(name="tmp", bufs=1) as tmp_pool,
        tc.tile_pool(name="qkt", bufs=2) as qkt_pool,
        tc.tile_pool(name="vaug", bufs=2) as vaug_pool,
        tc.tile_pool(name="pt", bufs=6) as pt_pool,
        tc.tile_pool(name="small", bufs=4) as small_pool,
        tc.tile_pool(name="oacc", bufs=1) as oacc_pool,
        tc.tile_pool(name="stage", bufs=2, space="PSUM") as stage_pool,
        tc.tile_pool(name="ypsum", bufs=4, space="PSUM") as ypsum_pool,
    ):
        xbf = xbf_pool.tile([128, NT, CIN], BF16)

        def load_x_chunk(third, half):
            c0 = third * E + half * (E // 2)
            return nc.gpsimd.dma_start(
                out=xbf[:, :, c0 : c0 + E // 2],
                in_=x_in[:, c0 : c0 + E // 2].rearrange(
                    "(a p) c -> p a c", p=128
                ),
            )

        # startup: q/k/v half-0 chunks first (q,k feed the critical mixes)
        load_x_chunk(0, 0)
        load_x_chunk(1, 0)
        load_x_chunk(2, 0)

        # ---- constants: strict-upper selector, MASK_NEG * I, ones row ---
        ustrict = consts.tile([128, 128], BF16)
        nc.gpsimd.memset(ustrict, 1.0)
        nc.gpsimd.affine_select(
            out=ustrict, in_=ustrict, compare_op=ALU.is_gt, fill=0.0,
            base=0, pattern=[[1, 128]], channel_multiplier=-1,
        )
        negi = consts.tile([128, 128], BF16)
        nc.gpsimd.memset(negi, 0.0)
        nc.gpsimd.affine_select(
            out=negi, in_=negi, compare_op=ALU.not_equal, fill=MASK_NEG,
            base=0, pattern=[[-1, 128]], channel_multiplier=1,
        )
        onesf = consts.tile([128, 8], F32)
        nc.gpsimd.memset(onesf, 1.0)

        oacc = oacc_pool.tile([128, NT, E], F32)

        state = {}

        # weight order in W: for cfg ci, e in (384, 576, 768): W[3*ci + idx]
        def mix_config(oi):
            """Generator. Emits DVE mixing + bounce/transpose DMAs for one
            config (order index oi), yielding between DVE ops.  Yields
            "ready" (oi==0 only) once attention may start."""
            ci = CFG_ORDER[oi]
            h = N_HEAD_LIST[ci]
            hd = E // h
            pw = _pw(h)
            h2 = h // 2
            e_list = [(2, 768, hd), (1, 576, 576 // h), (0, 384, 384 // h)]
            ndt = h * pw // 128
            ndt2 = ndt // 2

            qkt = []
            vaug = vaug_pool.tile([128, NT, h, hd + 1], BF16, tag="vaug")
            for tensor_idx in range(2):
                tl = qkt_pool.tile(
                    [128, ndt, T], BF16, tag="qkt", bufs=4,
                    name=f"qkt{ci}{tensor_idx}",
                )
                qkt.append(tl)
            tmp = tmp_pool.tile([128, NT, 288], BF16, tag="tmp")
            tmpb = tmp_pool.tile([128, NT, 288], BF16, tag="tmpb")
            state[ci] = (qkt, vaug)

            def mix_into(out_ap, xsrc, tmps):
                """Yields after each DVE op. out_ap(hde) is the dest slice,
                xsrc(e, hde) the source slice for mixture term e."""
                for idx, (k, e, hde) in enumerate(e_list):
                    w = float(W[3 * ci + k])
                    in0 = xsrc(e, hde)
                    if idx == 0:
                        nc.vector.tensor_scalar(
                            out_ap(hde), in0, w, None, ALU.mult
                        )
                        yield
                    else:
                        tview = tmps[idx % len(tmps)].rearrange(
                            "p a (h d) -> p a h d", h=h2
                        )
                        tv = tview[:, :, :, 0:hde]
                        nc.vector.tensor_scalar(tv, in0, w, None, ALU.mult)
                        yield
                        nc.vector.tensor_tensor(
                            out_ap(hde), tv, out_ap(hde), ALU.add
                        )
                        yield

            k0_transp = [None]
            for half in range(2):
                if oi == 0 and half == 1:
                    # half-1 x loads: hold behind the K half-0 transpose so
                    # they don't cut ahead on the FIFO DMA device
                    for third in range(3):
                        ld = load_x_chunk(third, 1)
                        if k0_transp[0] is not None:
                            add_dep_helper(
                                ld.ins, k0_transp[0].ins, sync=True,
                                reason="x h1 after startup transposes",
                            )
                hsl = slice(half * h2, (half + 1) * h2)
                for tensor_idx in range(2):
                    base = tensor_idx * E
                    nat = nat_pool.tile([128, NT, h2, pw], BF16, tag="nat")
                    if pw > hd:
                        nc.vector.memset(nat[:, :, :, hd:pw], 0.0)

                    def xsrc(e, hde, base=base, half=half):
                        sl = xbf[
                            :, :,
                            base + half * (e // 2)
                            : base + (half + 1) * (e // 2),
                        ]
                        return sl.rearrange("p a (h d) -> p a h d", h=h2)

                    def out_ap(hde, nat=nat):
                        return nat[:, :, :, 0:hde]

                    for _ in mix_into(out_ap, xsrc, (tmp, tmpb)):
                        yield

                    # bounce to DRAM + one 3D transpose read.  The startup-
                    # critical K half-0 chain of the first config rides the
                    # ACT ring (idle until the first exp, which depends on
                    # this transpose anyway) so it doesn't serialize behind
                    # the Q chain on SP; everything else stays on SP where
                    # DMA waits can't stall the exp stream's sequencer.
                    startup_k = oi == 0 and half == 0 and tensor_idx == 1
                    eng = nc.scalar if startup_k else nc.sync
                    w0 = half * h2 * pw
                    wr = eng.dma_start(
                        out=qk_dram[ci][tensor_idx][
                            :, w0 : w0 + h2 * pw
                        ].rearrange("(a p) w -> p a w", p=128),
                        in_=nat[:, :, :, :],
                    )
                    rd = eng.dma_start(
                        out=qkt[tensor_idx][
                            :, half * ndt2 : (half + 1) * ndt2, :
                        ],
                        in_=qk_dram[ci][tensor_idx][:, w0 : w0 + h2 * pw],
                        transpose=True,
                    )
                    add_dep_helper(
                        rd.ins, wr.ins, sync=True, reason="dram bounce raw"
                    )
                    if half == 0 and tensor_idx == 1:
                        k0_transp[0] = rd
                        if oi == 0:
                            # attention can start: Q0/K0 transposes queued.
                            # V0 mixes next as the first pumped pieces, so
                            # the scheduler can't slot them ahead of the K0
                            # chain's DVE deps.
                            yield "ready"
                    yield

                # V_aug for this half
                nc.gpsimd.memset(vaug[:, :, hsl, hd : hd + 1], 1.0)

                def vsrc(e, hde, half=half):
                    sl = xbf[
                        :, :,
                        2 * E + half * (e // 2)
                        : 2 * E + (half + 1) * (e // 2),
                    ]
                    return sl.rearrange("p a (h d) -> p a h d", h=h2)

                def vout(hde, hsl=hsl):
                    return vaug[:, :, hsl, 0:hde]

                for _ in mix_into(vout, vsrc, (tmp, tmpb)):
                    yield

        def attention():
            """Single software-pipelined job stream across all 3 configs."""
            prev = [None]  # carried (emit_fn, tk, g, ptl) across passes

            for oi, ci in enumerate(CFG_ORDER):
                if oi > 0:
                    yield ("cfg", oi)
                h = N_HEAD_LIST[ci]
                hd = E // h
                h2 = h // 2
                scale = 1.0 / math.sqrt(hd)
                dchunks = _dchunks(h)
                qkt, vaug = state[ci]
                qt, kt = qkt

                for hf in range(2):
                    for s in range(2):
                        ntk = 4 * s + 4
                        pheads = list(range(hf * h2, (hf + 1) * h2))
                        nh = h2
                        groups = [
                            pheads[i : i + 2] for i in range(0, nh, 2)
                        ]
                        yts = [
                            ypsum_pool.tile(
                                [128, nh, hd + 1], F32, tag="y",
                                name=f"yt{ci}{s}{hf}{qt_}",
                            )
                            for qt_ in range(4)
                        ]
                        y_first = [None] * 4

                        def norm_qt(qt_, *, oi=oi, s=s, hf=hf, yts=yts,
                                    pheads=pheads, nh=nh, hd=hd):
                            tqg = 4 * s + qt_
                            rec = small_pool.tile([128, 6], F32, tag="rec")
                            # last pass: DVE is idle (all mixing done) and
                            # Pool serialization would stretch the tail
                            veng = nc.vector if (oi == 2 and hf == 1) \
                                else nc.gpsimd
                            veng.tensor_tensor(
                                rec[:, 0:nh], onesf[:, 0:nh],
                                yts[qt_][:, :, hd], ALU.divide,
                            )
                            for jp, head in enumerate(pheads):
                                dst = oacc[
                                    :, tqg, head * hd : head * hd + hd
                                ]
                                if oi == 0:
                                    veng.tensor_scalar(
                                        dst, yts[qt_][:, jp, 0:hd],
                                        rec[:, jp : jp + 1], None, ALU.mult,
                                    )
                                else:
                                    veng.scalar_tensor_tensor(
                                        out=dst,
                                        in0=yts[qt_][:, jp, 0:hd],
                                        scalar=rec[:, jp : jp + 1],
                                        in1=dst,
                                        op0=ALU.mult,
                                        op1=ALU.add,
                                    )
                            if oi == 2 and hf == 1:
                                # this query tile is final: stream out
                                nc.sync.dma_start(
                                    out=out_d[tqg * 128 : (tqg + 1) * 128, :],
                                    in_=oacc[:, tqg, :],
                                )

                        def emit_pv(tk, g, ptl, *, s=s, hf=hf, nh=nh, hd=hd,
                                    yts=yts, y_first=y_first, vaug=vaug,
                                    groups=groups, norm_qt=norm_qt):
                            for qt_ in range(4):
                                qtg = 4 * s + qt_
                                if qtg < tk:
                                    continue
                                for j, head in enumerate(g):
                                    jp = head - hf * nh
                                    is_start = (
                                        tk == 0 and y_first[qt_] is None
                                    )
                                    mm = nc.tensor.matmul(
                                        out=yts[qt_][:, jp, :],
                                        lhsT=ptl[
                                            :, j, qt_ * 128 : (qt_ + 1) * 128
                                        ],
                                        rhs=vaug[:, tk, head, :],
                                        start=is_start,
                                        stop=(tk == qtg and jp == nh - 1),
                                    )
                                    if is_start:
                                        y_first[qt_] = mm
                                    elif tk == 0:
                                        add_dep_helper(
                                            mm.ins,
                                            y_first[qt_].ins,
                                            reason="psum zero-region order",
                                        )
                            if g is groups[-1] and 0 <= tk - 4 * s < 4:
                                norm_qt(tk - 4 * s)

                        for tk in range(ntk):
                            lo = max(0, tk * 128 - s * 512)
                            diag = tk >= 4 * s
                            dlo = tk * 128 - s * 512
                            for g in groups:
                                stage = stage_pool.tile(
                                    [128, 2, 512], F32, tag="stage"
                                )
                                for j, head in enumerate(g):
                                    chunks = dchunks[head]
                                    n_mm = len(chunks) + (1 if diag else 0)
                                    for mi, (a, b) in enumerate(chunks):
                                        nc.tensor.matmul(
                                            out=stage[:, j, lo:512],
                                            lhsT=kt[
                                                a % 128 : a % 128 + (b - a),
                                                a // 128,
                                                tk * 128 : (tk + 1) * 128,
                                            ],
                                            rhs=qt[
                                                a % 128 : a % 128 + (b - a),
                                                a // 128,
                                                s * 512 + lo : (s + 1) * 512,
                                            ],
                                            start=(mi == 0),
                                            stop=(mi == n_mm - 1),
                                        )
                                    if diag:
                                        nc.tensor.matmul(
                                            out=stage[:, j, dlo : dlo + 128],
                                            lhsT=ustrict[:, :],
                                            rhs=negi[:, :],
                                            start=False,
                                            stop=True,
                                        )
                                ptl = pt_pool.tile(
                                    [128, 2, 512], BF16, tag="pt"
                                )
                                nc.scalar.activation(
                                    out=ptl[:, 0:2, lo:512],
                                    in_=stage[:, 0:2, lo:512],
                                    func=ACTF.Exp,
                                    scale=scale,
                                )
                                if prev[0] is not None:
                                    pfn, ptk, pg, pptl = prev[0]
                                    pfn(ptk, pg, pptl)
                                prev[0] = (emit_pv, tk, g, ptl)
                                yield
            if prev[0] is not None:
                pfn, ptk, pg, pptl = prev[0]
                pfn(ptk, pg, pptl)

        # ---- driver: startup mix, then attention with mix pumping ------
        gens = deque([(oi, mix_config(oi)) for oi in range(3)])
        g0 = gens[0][1]
        while True:
            if next(g0) == "ready":
                break

        def pump(n):
            for _ in range(n):
                while gens:
                    try:
                        next(gens[0][1])
                        break
                    except StopIteration:
                        gens.popleft()
                else:
                    return

        def drain_through(oi):
            while gens and gens[0][0] <= oi:
                try:
                    next(gens[0][1])
                except StopIteration:
                    gens.popleft()

        for item in attention():
            if isinstance(item, tuple) and item[0] == "cfg":
                drain_through(item[1])
            else:
                pump(2)
        while gens:
            try:
                next(gens[0][1])
            except StopIteration:
                gens.popleft()


_PROGRAM_CACHE = {}


def _get_program(W):
    key = np.asarray(W, dtype=np.float32).tobytes()
    if key not in _PROGRAM_CACHE:
        _PROGRAM_CACHE[key] = _build_program(np.asarray(W, dtype=np.float32))
    return _PROGRAM_CACHE[key]


def kernel(x, weights):
    """x: [8, 1024, 2304] f32; weights: [9] f32 -> [8, 1024, 768] f32."""
    x = np.asarray(x, dtype=np.float32)
    weights = np.asarray(weights, dtype=np.float32)
    assert x.shape == (N_CORES, T, CIN), x.shape
    nc = _get_program(weights)
    in_maps = [{"x": np.ascontiguousarray(x[c])} for c in range(N_CORES)]
    res = run_bass_kernel_spmd(nc, in_maps, list(range(N_CORES)))
    return np.stack([res.results[c]["out"] for c in range(N_CORES)], axis=0)
